# revision 35
# baseline (speedup 1.0000x reference)
"""Trainium2 Bass kernel for the DFS-Mixer style-attention module.

Computation (per batch b):
    dot[k,c]  = sum_hw CT[c,hw] * CR[k,c,hw]
    norm[k,c] = sqrt(sum_hw CR[k,c,hw]^2)
    w[.,c]    = softmax_k(2 * dot[.,c] / norm[.,c])
    out[c,hw] = sum_k IR[k,c,hw] * w[k,c]

Sharding: data-parallel over batch B=8 across the 8 NeuronCores (one b per
core, no cross-core communication).

The module is DMA-bound (reads all of IR and CR once, tiny compute per
byte), so inputs are cast to bf16 on the host: per-core traffic drops from
75.5 MB (fp32) to 37.8 MB, which halves the DMA-roofline floor.  The
rel-err budget (2e-2) dwarfs bf16's ~0.4 % element error; all reductions
accumulate in fp32.

STREAMING SOFTMAX (no barrier): softmax is e_k / sum(e_k), so instead of
waiting for all K dots to subtract the max, use a CONSTANT logit shift:
e_k = exp(2*dot_k/norm_k - 8) (|2*sim| <= ~8.2 on N(0,1) data, so e stays
in [e-16, e^0.2] -- comfortably inside fp32/bf16 range), accumulate
u = sum_k e_k * IR_k in PSUM as soon as each k's dot is ready, and fold
the 1/sum(e) normalization into the per-partition `scale` operand of the
PSUM-drain copy (zero extra cost).  This removes the softmax barrier that
otherwise serializes all of c-tile t's matmuls behind the last of its K
dot-reductions.

Per-core layout: C=256 as 2 c-tiles x 128 SBUF partitions, HW=4096 free.
Per step s = (t, k):
  sync q:   DMA CR[k, ct] -> crt [128, 4096] bf16 (8 KB rows); the CR
            stream runs two steps AHEAD of the IR stream so its last
            weight chain lands before the tail IR tiles do
  gpsimd q: DMA IR[k', ct'] (k' permuted so the LAST IR tile is k=0,
            whose weight chain completed long before -- short tail)
  ACT:      Square(crt 2048-half) x2, accum -> dA0/dA1 cols (1 elem/cyc;
            2048-wide ops measure faster per element than 4096-wide)
  DVE:      fused scalar_tensor_tensor CR*CT x2, accum -> dD0/dD1 cols
            (no DVE op pairs a double-pumped multiply with a fused
            reduction, so the 1x STT at 2287 ns/half is the cheapest dot;
            tensor_reduce measures 1x as well)
  chain:    per (k-1, k) PAIR, spread over three steps so each ACT<->DVE
            handoff has a full step of issue distance: [A] DVE folds the
            half-accumulators ([P, 2] ops cost the same fixed ~1 us as
            [P, 1] -- pairing halves the small-op count), ACT Ln(sq) and
            rn = Exp(-ln(sq)/2 + ln2) = 2/sqrt(sq) (Ln/Exp/Square/Copy
            share ONE activation-table set, natural_log_exp_and_others,
            so ACT never reloads tables; a Sqrt-based chain cost 28
            reloads x 1.3 us); [B] DVE z = dot*rn, ACT e = Exp(z - 8);
            [C] DVE wm_k = diag(e_k) via the ones-diagonal mask
  lag 5:    PE: 8 bf16 diag matmuls into PSUM acc halves (start at the
            first k in IR order, stop at the last)
PSUM: two [128, 2048] fp32 halves per c-tile (4 banks each) on rotating
per-half tags, so c-tile 1's accumulation begins as each of c-tile 0's
halves drains.  Drains: ACT Copy with scale=1/sum(e) -> bf16 staging ->
scalar-queue DMA (the last tile arrives as quarters and drains alternate
ACT/DVE to shorten the tail).  Output is bf16, upcast on the host.
"""

import os
import sys

import numpy as np


def _import_concourse():
    try:
        import concourse.bass  # noqa: F401
    except ImportError:
        for p in ("/opt/trn_rl_repo", "/root/.axon_site/_ro/trn_rl_repo"):
            if os.path.isdir(p) and p not in sys.path:
                sys.path.insert(0, p)
        import concourse.bass  # noqa: F401


_import_concourse()

import ml_dtypes  # noqa: E402

import concourse.bass as bass  # noqa: E402
import concourse.mybir as mybir  # noqa: E402
from concourse import tile  # noqa: E402
from concourse.bass_utils import run_bass_kernel_spmd  # noqa: E402
from concourse.vector_clock import ScopedClock, VectorClock  # noqa: E402


def _split_multiwait_bir(bir: bytes) -> bytes:
    """The neuronxcc walrus in this container encodes at most ONE sync-wait
    per instruction; Tile emits several.  Hoist extra waits onto same-engine
    NoOp instructions inserted immediately before the original instruction
    (engines execute in order, so waiting earlier on the same engine is
    semantically identical).  Sem *updates* are left untouched (a DMA's
    completion-inc cannot move to a sequencer NoOp)."""
    import json

    j = json.loads(bir)
    ctr = 0
    for f in j.get("functions", []):
        for bb in f.get("blocks", []):
            out_insts = []
            for ins in bb.get("instructions", []):
                si = ins.get("sync_info")
                waits = (si or {}).get("on_wait") or []
                if len(waits) > 1:
                    for w in waits[:-1]:
                        ctr += 1
                        nop = {
                            "engine": ins["engine"],
                            "ins": [],
                            "outs": [],
                            "name": f"waitsplit-{ctr}",
                            "opcode": "NoOp",
                            "sync_info": {"on_update": [], "on_wait": [w]},
                        }
                        if "debug" in ins:
                            nop["debug"] = ins["debug"]
                        out_insts.append(nop)
                    si["on_wait"] = [waits[-1]]
                out_insts.append(ins)
            bb["instructions"] = out_insts
    return json.dumps(j).encode()


_orig_to_json_bytes = bass.Bass.to_json_bytes


def _patched_to_json_bytes(self, *a, **kw):
    return _split_multiwait_bir(_orig_to_json_bytes(self, *a, **kw))


bass.Bass.to_json_bytes = _patched_to_json_bytes


def _patched_drain_and_barrier(self, tick_clock, wait_clock):
    # Stock TileContext exit emits one Drain waiting on every used semaphore,
    # which this walrus rejects ("Too many sync wait commands").  Emit one
    # Drain per semaphore instead.
    gc = tick_clock.global_clock
    n = len(gc)
    nonzero = [p for p in range(n) if gc[p] > 0] or [0]
    for p in nonzero:
        d = self.nc.sync.drain()
        vec = [gc[q] if q == p else 0 for q in range(n)]
        wait_clock.add_sem_waits(d.ins, ScopedClock({None: VectorClock(vec)}))
    self.nc.all_engine_barrier()
    popped = self.nc._tile_sem_poison_stack.pop()
    assert popped is self._sem_poison
    self.nc.clear_and_free_semaphores(list(self.sems.allocated().values()))
    self.nc.all_engine_barrier()


tile.TileContext._drain_and_barrier = _patched_drain_and_barrier

FP = mybir.dt.float32
BF = mybir.dt.bfloat16
B, K, C, H, W = 8, 8, 256, 64, 64
HW = H * W
P = 128                 # SBUF partitions
NCT = C // P            # 2 c-tiles per core
MMN = 512               # moving free dim per matmul (= one PSUM bank of f32)
HN = 2048               # PSUM accumulator half width (4 banks of f32)
QN = 1024               # drain quarter width
HH = 2048               # consumer half-tile width
NSTEP = NCT * K         # 16 (t, k) steps
# IR consumption order within a c-tile: k=0 last, so the final IR tile's
# weight chain is long finished when its bytes arrive (short tail).
PI = [1, 2, 3, 4, 5, 6, 7, 0]

_AF = mybir.ActivationFunctionType
_OP = mybir.AluOpType
_X = mybir.AxisListType.X


def build_nc() -> bass.Bass:
    nc = bass.Bass()
    IR = nc.declare_dram_parameter("IR", [K, C, HW], BF, isOutput=False)
    CR = nc.declare_dram_parameter("CR", [K, C, HW], BF, isOutput=False)
    CT = nc.declare_dram_parameter("CT", [C, HW], BF, isOutput=False)
    EYE = nc.declare_dram_parameter("EYE", [P, P], FP, isOutput=False)
    OUT = nc.declare_dram_parameter("OUT", [C, HW], BF, isOutput=True)

    with tile.TileContext(nc) as tc:
        with (
            tc.tile_pool(name="ctp", bufs=1) as ct_pool,
            tc.tile_pool(name="crp", bufs=8) as cr_pool,
            tc.tile_pool(name="irp", bufs=6) as ir_pool,
            tc.tile_pool(name="ir7", bufs=4) as ir7_pool,
            tc.tile_pool(name="obp", bufs=4) as ob_pool,
            tc.tile_pool(name="snk", bufs=1) as sink_pool,
            tc.tile_pool(name="sml", bufs=1) as small,
            tc.tile_pool(name="psp", bufs=1, space="PSUM") as psum_pool,
        ):
            # Dead destinations for phase-1 elementwise outputs (only the
            # accum_out side-outputs are live).  One per engine so ACT and
            # DVE never serialize on a WAW.
            sinka = sink_pool.tile([P, HH], BF, name="sinka")
            sinkd = sink_pool.tile([P, HH], BF, name="sinkd")

            # Diagonal ones mask, DMA'd in as a constant (an
            # affine_select build runs on the slow Q7 gpsimd engine and
            # held up the entry barrier / first loads).
            mask = small.tile([P, P], FP, name="mask")
            nc.scalar.dma_start(out=mask[:], in_=EYE[:, :])
            neg8 = small.tile([P, 1], FP, name="neg8")
            nc.vector.memset(neg8[:], -8.0)
            mhalf = small.tile([P, 1], FP, name="mhalf")
            nc.vector.memset(mhalf[:], -0.5)
            ln2 = small.tile([P, 1], FP, name="ln2")
            nc.vector.memset(ln2[:], 0.6931471805599453)
            # Content-target features stay resident in SBUF (reused by all
            # k).  Issued from the scalar queue so they overlap the sync
            # queue's first CR issues.
            ct_tiles = []
            for t in range(NCT):
                ctt = ct_pool.tile([P, HW], BF, name=f"ct{t}", tag=f"ct{t}")
                nc.scalar.dma_start(out=ctt[:], in_=CT[t * P:(t + 1) * P, :])
                ct_tiles.append(ctt)

            # Per-(t, engine[, half]) reduction accumulators, single-writer
            # each so the engines never cross-serialize on a shared tile.
            dA0, dA1, dD0, dD1, Es, rss = [], [], [], [], [], [None] * NCT
            for t in range(NCT):
                dA0.append(small.tile([P, K], FP, name=f"dA0{t}"))
                dA1.append(small.tile([P, K], FP, name=f"dA1{t}"))
                dD0.append(small.tile([P, K], FP, name=f"dD0{t}"))
                dD1.append(small.tile([P, K], FP, name=f"dD1{t}"))
                Es.append(small.tile([P, K], FP, name=f"E{t}"))

            wms = {}          # (t, k) -> [128, 128] bf16 diag(e) tile
            irts = {}         # (t, k) -> IR tile (or list of quarters)
            acch = {}         # (t, h) -> PSUM [128, 2048] accumulator half

            h0 = slice(0, HH)
            h1 = slice(HH, 2 * HH)

            def load_and_reduce(t, k):
                """CR tile in; ACT squares both 2048-halves (measured faster
                per element than one 4096-wide op); DVE does both fused
                CR*CT dot halves (STT never double-pumps, but neither does
                tensor_reduce, so the fused 1x op is the cheapest dot).
                fp32 column accumulators, one per engine/half."""
                cs = slice(t * P, (t + 1) * P)
                crt = cr_pool.tile([P, HW], BF, name="crt", tag="cr")
                nc.sync.dma_start(out=crt[:], in_=CR[k, cs, :])
                nc.scalar.activation(
                    out=sinka[:], in_=crt[:, h0], func=_AF.Square,
                    accum_out=dA0[t][:, k:k + 1],
                )
                nc.scalar.activation(
                    out=sinka[:], in_=crt[:, h1], func=_AF.Square,
                    accum_out=dA1[t][:, k:k + 1],
                )
                nc.vector.scalar_tensor_tensor(
                    out=sinkd[:], in0=crt[:, h0], scalar=1.0,
                    in1=ct_tiles[t][:, h0],
                    op0=_OP.mult, op1=_OP.mult,
                    accum_out=dD0[t][:, k:k + 1],
                )
                nc.vector.scalar_tensor_tensor(
                    out=sinkd[:], in0=crt[:, h1], scalar=1.0,
                    in1=ct_tiles[t][:, h1],
                    op0=_OP.mult, op1=_OP.mult,
                    accum_out=dD1[t][:, k:k + 1],
                )

            def load_ir(t, k):
                cs = slice(t * P, (t + 1) * P)
                if (t, k) == (NCT - 1, PI[-1]):
                    # Very last IR tile arrives as quarters so the tail
                    # after the final byte is two short matmuls + drain.
                    qs_tiles = []
                    for q in range(HW // QN):
                        qs = slice(q * QN, (q + 1) * QN)
                        ir7 = ir7_pool.tile([P, QN], BF, name="ir7", tag="ir7")
                        nc.gpsimd.dma_start(out=ir7[:], in_=IR[k, cs, qs])
                        qs_tiles.append(ir7)
                    irts[(t, k)] = qs_tiles
                else:
                    irt = ir_pool.tile([P, HW], BF, name="irt", tag="ir")
                    nc.gpsimd.dma_start(out=irt[:], in_=IR[k, cs, :])
                    irts[(t, k)] = irt

            chain_state = {}

            def chain_a(t, k):
                """Stage A for the (k-1, k) pair (runs at odd k): fold the
                per-half fp32 accumulators ([P, 2] ops cost the same ~1 us
                fixed overhead as [P, 1], so pairing halves the small-op
                count) and start the table-free rsqrt: Ln and Exp live in
                the SAME activation-table set as Square and Copy
                (natural_log_exp_and_others), so ACT never reloads tables.
                The chain is spread over three schedule steps (A/B/C) so
                every cross-engine handoff has a full step of issue
                distance and neither in-order queue ever stalls on it."""
                k2 = slice(k - 1, k + 1)
                dsum = small.tile([P, 2], FP, name=f"du{t}{k}")
                nc.vector.tensor_add(dsum[:], dD0[t][:, k2], dD1[t][:, k2])
                ssum = small.tile([P, 2], FP, name=f"su{t}{k}")
                nc.vector.tensor_add(ssum[:], dA0[t][:, k2], dA1[t][:, k2])
                lsq = small.tile([P, 2], FP, name=f"ls{t}{k}")
                nc.scalar.activation(lsq[:], ssum[:], func=_AF.Ln)
                # rn = exp(-ln(sq)/2 + ln 2) = 2/sqrt(sq): the logit's 2x
                # factor rides the unused bias operand for free.
                rn = small.tile([P, 2], FP, name=f"rn{t}{k}")
                nc.scalar.activation(
                    rn[:], lsq[:], func=_AF.Exp, scale=mhalf[:, 0:1],
                    bias=ln2[:, 0:1],
                )
                chain_state[(t, k)] = (dsum, rn)

            def chain_b(t, k):
                """Stage B: z = (2*dot/norm); e = exp(z - 8) into E cols."""
                k2 = slice(k - 1, k + 1)
                dsum, rn = chain_state[(t, k)]
                z = small.tile([P, 2], FP, name=f"z{t}{k}")
                nc.vector.tensor_mul(z[:], dsum[:], rn[:])
                nc.scalar.activation(
                    Es[t][:, k2], z[:], func=_AF.Exp, bias=neg8[:, 0:1]
                )

            def chain_c(t, k):
                """Stage C: the pair's diag(e) weight matrices, plus the
                1/sum(e) drain scale once the c-tile's E row completes."""
                for kk in (k - 1, k):
                    wm = small.tile([P, P], BF, name=f"wm{t}{kk}")
                    nc.vector.tensor_scalar_mul(
                        wm[:], mask[:], Es[t][:, kk:kk + 1]
                    )
                    wms[(t, kk)] = wm
                if k == K - 1:
                    ssum = small.tile([P, 1], FP, name=f"se{t}")
                    nc.vector.reduce_sum(ssum[:], Es[t][:], axis=_X)
                    rs = small.tile([P, 1], FP, name=f"rs{t}")
                    nc.vector.reciprocal(rs[:], ssum[:])
                    rss[t] = rs

            def drain(t, q, on_dve=False):
                """Copy a finished PSUM quarter to bf16 staging with the
                1/sum(e) softmax normalization folded into the copy's
                per-partition scale; stream out on the scalar queue."""
                cs = slice(t * P, (t + 1) * P)
                h, qq = divmod(q, 2)
                src = acch[(t, h)][:, qq * QN:(qq + 1) * QN]
                ob = ob_pool.tile([P, QN], BF, name="ob", tag="ob")
                if on_dve:
                    nc.vector.tensor_scalar_mul(ob[:], src, rss[t][:, 0:1])
                else:
                    nc.scalar.activation(
                        ob[:], src, func=_AF.Copy, scale=rss[t][:, 0:1]
                    )
                nc.scalar.dma_start(
                    out=OUT[cs, q * QN:(q + 1) * QN], in_=ob[:]
                )

            def matmuls(t, k, first, last):
                """Fold e_k * IR_k into the PSUM halves (bf16 diag matmul,
                1 cyc/row).  At the last k, close the banks and drain."""
                if first:
                    # One rotating buffer per half-tag: c-tile 1's half h
                    # takes over c-tile 0's banks once its drains complete.
                    for h in range(2):
                        acch[(t, h)] = psum_pool.tile(
                            [P, HN], FP, name=f"acc{t}{h}", tag=f"acch{h}"
                        )
                wm = wms[(t, k)]
                src = irts[(t, k)]
                if isinstance(src, list):
                    # Quartered final tile: drain each quarter as its bank
                    # closes, alternating the copy engine to shorten the
                    # end-of-kernel tail.
                    for q in range(HW // QN):
                        h, qq = divmod(q, 2)
                        for jj in range(QN // MMN):
                            col = qq * QN + jj * MMN
                            nc.tensor.matmul(
                                acch[(t, h)][:, col:col + MMN],
                                wm[:],
                                src[q][:, jj * MMN:(jj + 1) * MMN],
                                start=first,
                                stop=last,
                            )
                        drain(t, q, on_dve=(q % 2 == 1))
                else:
                    for j in range(HW // MMN):
                        h, col = divmod(j * MMN, HN)
                        nc.tensor.matmul(
                            acch[(t, h)][:, col:col + MMN],
                            wm[:],
                            src[:, j * MMN:(j + 1) * MMN],
                            start=first,
                            stop=last,
                        )
                    if last:
                        # Mid-stream drains all ride ACT (its queue has
                        # slack; DVE is the cadence-limiting engine).
                        for q in range(HW // QN):
                            drain(t, q)

            # ---- Fully streamed schedule ----
            # Step s: stage B for the pair ending at s-2 leads the body, so
            # its ACT Exp completes while DVE chews the step's dots and the
            # trailing stage C (DVE wm builds, same pair) never stalls on
            # the cross-engine handoff.  IR loads are skewed three steps
            # behind CR (the CR stream finishes first, so the last weight
            # chain never gates the tail matmuls); matmuls run at lag 4.
            for s in range(NSTEP + 5):
                e_b = s - 2
                if 0 <= e_b < NSTEP:
                    t, k = divmod(e_b, K)
                    if k % 2 == 1:
                        chain_b(t, k)
                if s < NSTEP:
                    t, k = divmod(s, K)
                    load_and_reduce(t, k)
                e_a = s - 1
                if 0 <= e_a < NSTEP:
                    t, k = divmod(e_a, K)
                    if k % 2 == 1:
                        chain_a(t, k)
                if 0 <= e_b < NSTEP:
                    t, k = divmod(e_b, K)
                    if k % 2 == 1:
                        chain_c(t, k)
                if 0 <= s - 3 < NSTEP:
                    t, i = divmod(s - 3, K)
                    load_ir(t, PI[i])
                if 0 <= s - 4 < NSTEP:
                    t, i = divmod(s - 4, K)
                    matmuls(t, PI[i], first=(i == 0), last=(i == K - 1))

    return nc


_NC_CACHE = None


def _get_nc() -> bass.Bass:
    global _NC_CACHE
    if _NC_CACHE is None:
        _NC_CACHE = build_nc()
    return _NC_CACHE


def run(inputs: dict, trace: bool = False):
    """Shard over B, run on 8 cores, gather. Returns (output, BassKernelResults)."""
    bf16 = ml_dtypes.bfloat16
    ir = np.asarray(inputs["IR_features"], dtype=np.float32).astype(bf16)
    cr = np.asarray(inputs["CR_features"], dtype=np.float32).astype(bf16)
    ct = np.asarray(inputs["CT_feature"], dtype=np.float32).astype(bf16)
    assert ir.shape == (B, K, C, H, W) and cr.shape == (B, K, C, H, W)
    assert ct.shape == (B, C, H, W)

    eye = np.eye(P, dtype=np.float32)
    in_maps = [
        {
            "IR": np.ascontiguousarray(ir[b].reshape(K, C, HW)),
            "CR": np.ascontiguousarray(cr[b].reshape(K, C, HW)),
            "CT": np.ascontiguousarray(ct[b].reshape(C, HW)),
            "EYE": eye,
        }
        for b in range(B)
    ]
    res = run_bass_kernel_spmd(_get_nc(), in_maps, list(range(B)), trace=trace)
    out = np.stack([res.results[b]["OUT"] for b in range(B)])
    return out.reshape(B, C, H, W).astype(np.float32), res


def kernel(**inputs) -> np.ndarray:
    return run(inputs)[0]
